# revision 35
# baseline (speedup 1.0000x reference)
"""Trainium2 Bass kernel for AttnBlock (GroupNorm + QKV + NxN attention + proj + residual).

Contract: kernel(**inputs) takes the FULL unsharded inputs (as produced by
setup_inputs) and returns the FULL output, running on 8 NeuronCores via
bass_utils.run_bass_kernel_spmd.

Sharding: core i handles (batch b = i//4, query-shard s = i%4). The host
rotates x[b] by -s*1024 along the flattened spatial axis so the (identical)
SPMD program always treats columns 0:1024 as its query rows: attention and
GroupNorm are permutation-invariant over key positions, so only the output
column order matters, and out columns 0:1024 of the rotated problem are
exactly out[b][:, s*1024:(s+1)*1024] of the original.

Key layout decisions (fp8 DoubleRow everywhere):
  - GroupNorm is folded into the weights on the host: hn = x*s + t with
    per-(batch,channel) s,t from exact fp32 stats, so q = (wq*s)@x + wq@t+bq.
    The v-side constant wp@(wv@t+bv)+bp is added on the host.
  - x ships as fp8e4m3 (1MB/core) in DoubleRow layout [128p, 2half, 4096],
    chunked across both HWDGE rings; weights/biases on the gpsimd SWDGE ring
    (q/k/v weights fp8, proj weights fp16).
  - q/k/v production, scores S^T = K^T Q, PV, and the softmax denominator
    (ones-column stationary) are all single DoubleRow fp8 matmuls at 2x PE
    rate; fp32 PSUM accumulation throughout.
  - softmax needs no row max (|s| <= ~8): exp on ACT per (128,1024)
    score-pair tile, output e4m3 as exp(s/16 - OFS) so values stay under
    TRN-fp8's 240 cap (e^-OFS cancels in wout/den on the host).
  - query halves nh-outer (512 cols); PSUM: 2x (128,2,512) score slots
    (also warmup/q/proj) + 3-deep k/v/h ring + denominator bank = 8 banks.
  - schedule: phase1 = k/v production + nh=0 scores/exps (exp tiles buffered
    in SBUF, bufs=20), lagged one j behind k/v to hide the copy latency;
    phase2a = nh=0 PV/den backlog (skewed 3 iters behind) on the PE while
    ACT streams nh=1 exps back-to-back; phase2b = nh=1 PV/den (no ACT
    dependency left); nh=0 projection/output DMA overlaps phase2a/b.
  - outputs: unnormalized projection wout (fp16) + denominator den (fp32);
    host computes out = x + bpp + wout/den during unsharding.
"""

import numpy as np

C = 256
N = 4096  # spatial positions (16*16*16)
NSH = 1024  # query shard per core
NCORES = 8
EPS = 1e-6
SCALE = 1.0 / 16.0  # C ** -0.5
OFS = 2.5  # exp offset: ex = exp(s*SCALE - OFS), keeps e4m3 under 240

_CACHE = {}


def _build_program():
    import concourse.bass as bass
    import concourse.tile as tile
    from concourse import bacc, mybir

    F32 = mybir.dt.float32
    F16 = mybir.dt.float16
    F8 = mybir.dt.float8e4
    Alu = mybir.AluOpType
    Act = mybir.ActivationFunctionType
    DR = mybir.MatmulPerfMode.DoubleRow

    nc = bacc.Bacc("TRN2", target_bir_lowering=False, debug=False,
                   num_devices=NCORES)

    # x in fp8e4m3, DoubleRow layout: x8[p, h, n] = x[h*128+p, n]
    d_x8 = nc.dram_tensor("x8", [128, 2, N], F8, kind="ExternalInput").ap()
    # w8[p, h, :] = [wq'T | wk'T | wv'T] of channel-half h (GroupNorm-scaled)
    d_w8 = nc.dram_tensor("w8", [128, 2, 3 * C], F8, kind="ExternalInput").ap()
    d_wp = nc.dram_tensor("wp16", [2, 128, C], F16, kind="ExternalInput").ap()
    # qbc[p, {qb, kb}, half]: effective q/k biases (w@t + b), fp32
    d_qbc = nc.dram_tensor("qbc", [128, 2, 2], F32, kind="ExternalInput").ap()
    # outputs: unnormalized projection (host divides by den and adds residual)
    d_wout = nc.dram_tensor("wout", [2, 2, 128, 512], F16, kind="ExternalOutput").ap()
    d_den = nc.dram_tensor("den", [2, 512], F32, kind="ExternalOutput").ap()

    MCH = N // 128   # 32 key chunks
    NPAIR = MCH // 2  # 16 key-chunk pairs

    with tile.TileContext(nc) as tc:
        with (
            tc.tile_pool(name="persist", bufs=1) as P,
            tc.tile_pool(name="work", bufs=2) as W,
            tc.tile_pool(name="psum", bufs=1, space="PSUM") as PS,
        ):
            # ---- inputs: weights first on the HWDGE rings, then x half0;
            # x half1 on the gpsimd SWDGE ring (per-HWDGE-queue BW ~90GB/s)
            x8 = P.tile([128, 2, N], F8, tag="x8")
            for lo, hi in ((0, 512), (512, 1024), (1024, 2048),
                           (2048, 3072), (3072, 4096)):
                sl = slice(lo, hi)
                nc.sync.dma_start(out=x8[:, 0, sl], in_=d_x8[:, 0, sl])
                nc.scalar.dma_start(out=x8[:, 1, sl], in_=d_x8[:, 1, sl])
            w8 = P.tile([128, 2, 3 * C], F8, tag="w8")
            nc.gpsimd.dma_start(out=w8, in_=d_w8)
            wp16 = []
            for h in range(2):
                t = P.tile([128, C], F16, tag=f"wp{h}", name=f"wp{h}")
                nc.gpsimd.dma_start(out=t, in_=d_wp[h])
                wp16.append(t)
            qbc = P.tile([128, 2, 2], F32, tag="qbc")
            nc.gpsimd.dma_start(out=qbc, in_=d_qbc)

            ones16 = P.tile([128, 128], F16, tag="ones16")
            nc.vector.memset(ones16, 1.0)
            ones8 = P.tile([128, 2, 32], F8, tag="ones8")
            nc.vector.memset(ones8, 1.0)
            nofs = P.tile([128, 1], F32, tag="nofs")
            nc.vector.memset(nofs, -OFS)

            qb = [qbc[:, 0, h:h + 1] for h in range(2)]
            kb = [qbc[:, 1, h:h + 1] for h in range(2)]

            wpT = wp16

            # ---- PE warmup (HAM clock ramp): self-matmuls, then x chunks ----
            for j in range(4):
                wm = PS.tile([128, 128], F32, tag="st", bufs=2,
                             name=f"warmo_{j}")
                nc.tensor.matmul(wm, ones16, ones16)
            for j in range(2):
                wm = PS.tile([32, 512], F32, tag="st", bufs=2,
                             name=f"warm_{j}")
                nc.tensor.matmul(wm, ones8, x8[:, :, j * 512:(j + 1) * 512],
                                 perf_mode=DR)

            # ---- q (shard columns 0:1024) -> fp8 DoubleRow layout ----
            # qf8[nh][p, h, n] = q[h*128+p, nh*512+n]
            qf8 = [P.tile([128, 2, 512], F8, tag=f"qf{nh}", name=f"qf{nh}")
                   for nh in range(2)]

            def make_q(nh, ohs=(0, 1)):
                for oh in ohs:
                    qp = PS.tile([128, 512], F32, tag="st", bufs=2,
                                 name=f"qp{nh}{oh}")
                    sl = slice(nh * 512, (nh + 1) * 512)
                    nc.tensor.matmul(
                        qp, w8[:, :, oh * 128:(oh + 1) * 128],
                        x8[:, :, sl], perf_mode=DR)
                    nc.vector.tensor_scalar(
                        out=qf8[nh][:, oh, :], in0=qp, scalar1=qb[oh],
                        scalar2=None, op0=Alu.add)

            make_q(0)

            # ---- persistent fp8 k / vT ----
            # kf8[p, h, m] = k[h*128+p, m]
            kf8 = P.tile([128, 2, N], F8, tag="kf8")
            # vt[mp][p, i, o] = v[o, (2*mp+i)*128 + p]
            vt = [P.tile([128, 2, C], F8, tag=f"vt{mp}", name=f"vt{mp}")
                  for mp in range(NPAIR)]

            h_ps = {}
            den_ps = {}
            ex2 = {}

            def s_exp(nh, mp):
                """Score pair + exp into an SBUF-buffered fp8 tile."""
                st = PS.tile([128, 2, 512], F32, tag="st", bufs=2,
                             name=f"st{nh}_{mp}")
                for ii in range(2):
                    mc = 2 * mp + ii
                    nc.tensor.matmul(
                        st[:, ii, :],
                        kf8[:, :, mc * 128:(mc + 1) * 128],
                        qf8[nh], perf_mode=DR)
                e = W.tile([128, 2, 512], F8, tag="ex", bufs=24,
                           name=f"ex{nh}_{mp}")
                nc.scalar.activation(out=e, in_=st, func=Act.Exp,
                                     bias=nofs, scale=SCALE)
                ex2[(nh, mp)] = e

            def pv_den(nh, mp):
                """PV + denominator accumulation from the buffered exp tile."""
                if mp == 0:
                    h_ps[nh] = [PS.tile([128, 512], F32, tag="hkv", bufs=3,
                                        name=f"hps{nh}_{ch}")
                                for ch in range(2)]
                    den_ps[nh] = PS.tile([32, 512], F32, tag="den", bufs=1,
                                         name=f"den{nh}")
                e = ex2.pop((nh, mp))
                for ch in range(2):
                    nc.tensor.matmul(
                        h_ps[nh][ch],
                        vt[mp][:, :, ch * 128:(ch + 1) * 128],
                        e, perf_mode=DR,
                        start=(mp == 0), stop=(mp == NPAIR - 1))
                nc.tensor.matmul(
                    den_ps[nh], ones8, e, perf_mode=DR,
                    start=(mp == 0), stop=(mp == NPAIR - 1))

            hr = [[P.tile([128, 512], F16, tag=f"hr{nh}{ch}",
                          name=f"hr{nh}{ch}") for ch in range(2)]
                  for nh in range(2)]
            densb = [P.tile([1, 512], F32, tag=f"densb{nh}", name=f"densb{nh}")
                     for nh in range(2)]

            def finalize(nh):
                nc.vector.tensor_copy(out=hr[nh][0], in_=h_ps[nh][0])
                nc.scalar.copy(out=hr[nh][1], in_=h_ps[nh][1])

            def finalize_den(nh):
                nc.vector.tensor_copy(out=densb[nh], in_=den_ps[nh][0:1, :])
                nc.sync.dma_start(out=d_den[nh], in_=densb[nh])

            def proj(nh):
                for oh in range(2):
                    op = PS.tile([128, 512], F32, tag="st", bufs=2,
                                 name=f"op{nh}{oh}")
                    for ch in range(2):
                        nc.tensor.matmul(
                            op, wpT[ch][:, oh * 128:(oh + 1) * 128],
                            hr[nh][ch], start=(ch == 0), stop=(ch == 1))
                    osb = W.tile([128, 512], F16, tag="osb", bufs=2,
                                 name=f"osb{nh}{oh}")
                    if oh == 0:
                        nc.vector.tensor_copy(out=osb, in_=op)
                    else:
                        nc.scalar.copy(out=osb, in_=op)
                    eng = nc.gpsimd if (nh == 0 and oh == 1) else nc.sync
                    eng.dma_start(out=d_wout[nh, oh], in_=osb)

            # ---- phase 1: k/v production + nh=0 scores/exps (buffered) ----
            for j in range(8):
                msl = slice(j * 512, (j + 1) * 512)
                for oh in range(2):
                    kp = PS.tile([128, 512], F32, tag="hkv", bufs=3,
                                 name=f"kp{j}{oh}")
                    nc.tensor.matmul(
                        kp, w8[:, :, C + oh * 128:C + (oh + 1) * 128],
                        x8[:, :, msl], perf_mode=DR)
                    nc.vector.tensor_scalar(
                        out=kf8[:, oh, msl], in0=kp, scalar1=kb[oh],
                        scalar2=None, op0=Alu.add)
                for i in range(2):
                    mp = 2 * j + i
                    vp = PS.tile([128, 2, C], F32, tag="hkv", bufs=3,
                                 name=f"vp{mp}")
                    for ii in range(2):
                        mc = 2 * mp + ii
                        nc.tensor.matmul(
                            vp[:, ii, :],
                            x8[:, :, mc * 128:(mc + 1) * 128],
                            w8[:, :, 2 * C:3 * C], perf_mode=DR)
                    nc.vector.tensor_copy(out=vt[mp], in_=vp)
                if j == 0:
                    make_q(1, ohs=(0,))
                elif j == 6:
                    make_q(1, ohs=(1,))
                if j > 0:
                    for i in range(2):
                        s_exp(0, 2 * (j - 1) + i)
            for i in range(2):
                s_exp(0, 14 + i)

            # ---- phase 2a: nh=0 PV/den backlog + nh=1 scores/exps
            # (PV drain skewed 3 iters so the exp stream fills first) ----
            for mp in range(NPAIR):
                s_exp(1, mp)
                if mp >= 3:
                    pv_den(0, mp - 3)
            for mp in range(NPAIR - 3, NPAIR):
                pv_den(0, mp)
            finalize(0)
            proj(0)
            finalize_den(0)
            # ---- phase 2b: nh=1 PV/den backlog (no ACT left) ----
            for mp in range(NPAIR):
                pv_den(1, mp)
            finalize(1)
            proj(1)
            finalize_den(1)

    nc.compile()
    return nc


def _host_inputs(x, gamma, beta, wq, bq, wk, bk, wv, bv, wp, bp):
    """Build the per-core input maps (list of 8 dicts) + per-batch out bias."""
    f16 = np.float16
    f32 = np.float32
    xr = np.asarray(x, f32).reshape(2, C, N)

    # exact fp32 GroupNorm stats (32 groups of 8 channels)
    xg = xr.reshape(2, 32, 8 * N)
    mean = xg.mean(axis=2)
    var = xg.var(axis=2)
    rstd = 1.0 / np.sqrt(var + EPS)
    g32 = np.asarray(gamma, f32)
    b32 = np.asarray(beta, f32)
    s_bc = np.repeat(rstd, 8, axis=1) * g32[None, :]          # (2, C)
    t_bc = b32[None, :] - np.repeat(mean, 8, axis=1) * s_bc   # (2, C)

    wq32 = np.asarray(wq, f32)
    wk32 = np.asarray(wk, f32)
    wv32 = np.asarray(wv, f32)
    wp32 = np.asarray(wp, f32)
    bq32 = np.asarray(bq, f32)
    bk32 = np.asarray(bk, f32)
    bv32 = np.asarray(bv, f32)
    bp32 = np.asarray(bp, f32)

    import ml_dtypes
    f8 = ml_dtypes.float8_e4m3fn

    w8s, wps, qbcs, bpps = [], [], [], []
    for b in range(2):
        s = s_bc[b][None, :]
        w3 = np.concatenate(
            [(wq32 * s).T, (wk32 * s).T, (wv32 * s).T], axis=1)  # (C, 3C)
        w8s.append(np.ascontiguousarray(
            w3.reshape(2, 128, 3 * C).transpose(1, 0, 2)).astype(f8))
        wps.append(np.ascontiguousarray(
            wp32.T.reshape(2, 128, C)).astype(f16))
        qb_b = wq32 @ t_bc[b] + bq32
        kb_b = wk32 @ t_bc[b] + bk32
        qbc = np.stack([qb_b.reshape(2, 128), kb_b.reshape(2, 128)],
                       axis=0).transpose(2, 0, 1)             # (128, 2, 2)
        qbcs.append(np.ascontiguousarray(qbc).astype(f32))
        bpps.append(wp32 @ (wv32 @ t_bc[b] + bv32) + bp32)

    in_maps = []
    for core in range(NCORES):
        b, sdx = divmod(core, 4)
        xrot = np.roll(xr[b], -sdx * NSH, axis=1)
        x8 = np.ascontiguousarray(
            xrot.reshape(2, 128, N).transpose(1, 0, 2)).astype(f8)
        in_maps.append({
            "x8": x8,
            "w8": w8s[b],
            "wp16": wps[b],
            "qbc": qbcs[b],
        })
    return in_maps, bpps


def _gather(results, x, bpps):
    """Unshard: out = x + bpp + wout / den (division commutes with wp)."""
    xr = np.asarray(x, np.float32).reshape(2, C, N)
    out = np.empty((2, C, N), np.float32)
    for core in range(NCORES):
        b, sdx = divmod(core, 4)
        w4 = results[core]["wout"].astype(np.float32)   # (2nh, 2oh, 128, 512)
        den = results[core]["den"].astype(np.float32)   # (2nh, 512)
        wfull = np.empty((C, NSH), np.float32)
        for nh in range(2):
            for oh in range(2):
                wfull[oh * 128:(oh + 1) * 128, nh * 512:(nh + 1) * 512] = \
                    w4[nh, oh] / den[nh][None, :]
        sl = slice(sdx * NSH, (sdx + 1) * NSH)
        out[b, :, sl] = xr[b, :, sl] + bpps[b][:, None] + wfull
    return out.reshape(2, C, 16, 16, 16)


def kernel(x, gamma, beta, wq, bq, wk, bk, wv, bv, wp, bp):
    from concourse import bass_utils

    if "nc" not in _CACHE:
        _CACHE["nc"] = _build_program()
    nc = _CACHE["nc"]
    in_maps, bpps = _host_inputs(x, gamma, beta, wq, bq, wk, bk, wv, bv,
                                 wp, bp)
    res = bass_utils.run_bass_kernel_spmd(nc, in_maps, core_ids=list(range(NCORES)))
    return _gather(res.results, x, bpps)
